# revision 18
# baseline (speedup 1.0000x reference)
"""Fused SwiGLU MLP (gate/up/down) Trainium2 Bass kernel.

Problem: y = down( silu(x @ Wg^T) * (x @ Wu^T) ) with
  x  [B=2, S=2048, H=4096]  f32
  Wg [I=11008, H]           f32   (gate proj, [out,in])
  Wu [I=11008, H]           f32
  Wd [H, I]                 f32

Strategy: data-parallel over tokens across the 8 NeuronCores.
Each core gets T = 4096/8 = 512 tokens and the full (replicated) weights,
computing the entire MLP for its token shard.  No collectives; the host
just concatenates the 8 token shards.

Per-core PE floor (f32r/bf16 at 1 cycle/row, 2.4 GHz):
  gate/up: 86 subtiles x 64 matmuls x 512 rows = 2.818M rows
  down:    86 x 8 osc x 4 tt matmuls x 512 rows = 1.409M rows
  total 4.227M rows x 0.4167ns = 1.761 ms.

v2 changes vs the 2.126 ms v1:
  * bf16 weights + activations (same 1 cyc/row PE rate as f32r, half the
    HBM traffic: 281 MB vs 562 MB, ~2e-3 rel err vs 2e-4 -- both well
    inside the 2e-2 gate).
  * i-subtile-staggered gate/up: accumulate psg[s] fully (32 h matmuls),
    then psu[s], so only 1-2 PSUM banks accumulate at a time (v1 held
    all 8 banks, forcing the PE to drain-stall at every chunk boundary).
  * down-proj of chunk ci-1 interleaved between the gate/up groups of
    chunk ci, so the PE stream never waits on ACT/DVE: by the time a
    down block issues, its hm inputs closed >20us earlier.
  * no I-padding waste: 11008 = 86 x 128 exactly (v1 padded to 11264).
  * y written back per-osc during the trailing down blocks (no 8MB tail).

Device layouts (host-side numpy retile, every DMA partition-major):
  x    [4, 128, HS/4, T]     quarter q, partition p=h%128, h-sub, token
  wg/wu[NSUB, 128, HS, 128]  subtile s: [p=h%128, hs, i%128]
  wd   [NSUB, NO, 128, 512]  [p=i%128, o%512]
  y    [TT, 128, H]          y[tt*128+p, o]
"""

import math

import numpy as np

import concourse.bass as bass
import concourse.mybir as mybir
import concourse.tile as tile
from concourse import bacc
from concourse.bass_utils import run_bass_kernel_spmd

F32 = mybir.dt.float32
F32R = mybir.dt.float32r
BF16 = mybir.dt.bfloat16
P = 128
OCW = 512  # o-chunk width (one PSUM bank of f32)

# full-size problem constants
B, S, H, I = 2, 2048, 4096, 11008
NCORES = 8
T = (B * S) // NCORES  # 512 tokens per core


def _chunks(nsub, width=4):
    return [list(range(a, min(a + width, nsub))) for a in range(0, nsub, width)]


def build_nc(T, H, I_valid, wg_bufs=10, wd_bufs=12, hm_bufs=9, sg_bufs=2,
             mm_dt=BF16, use_silu=True):
    assert T % P == 0 and T <= 512
    HS = H // P          # h subtiles (contraction for gate/up)
    assert H % OCW == 0 and HS % 4 == 0
    NO = H // OCW        # o chunks
    TT = T // P          # token tiles
    NSUB = math.ceil(I_valid / P)
    IPAD = NSUB * P
    chunks = _chunks(NSUB)
    NCH = len(chunks)
    NXQ = 8 if HS % 8 == 0 else 4  # x split (finer => earlier first matmul)
    HQ = HS // NXQ       # h subtiles per x split tile
    WQ = HS // 4         # h subtiles per wg/wu split tile

    nc = bacc.Bacc("TRN2", target_bir_lowering=False, debug=False)
    x_d = nc.dram_tensor("x", [NXQ, P, HQ, T], mm_dt, kind="ExternalInput").ap()
    wg_d = nc.dram_tensor("wg", [NSUB, 4, P, WQ, P], mm_dt, kind="ExternalInput").ap()
    wu_d = nc.dram_tensor("wu", [NSUB, 4, P, WQ, P], mm_dt, kind="ExternalInput").ap()
    wd_d = nc.dram_tensor("wd", [NSUB, NO, P, OCW], mm_dt, kind="ExternalInput").ap()
    # y leaves in matmul dtype (bf16 halves the writeback tail); host upcasts
    y_dt = mm_dt if mm_dt == BF16 else F32
    y_d = nc.dram_tensor("y", [TT, NO, P, OCW], y_dt, kind="ExternalOutput").ap()

    with tile.TileContext(nc) as tc:
        with (
            tc.tile_pool(name="xp", bufs=1) as xp,
            tc.tile_pool(name="yp", bufs=1) as yp,
            tc.tile_pool(name="wgp", bufs=wg_bufs) as wgp,
            tc.tile_pool(name="wup", bufs=wg_bufs) as wup,
            tc.tile_pool(name="wdp", bufs=wd_bufs) as wdp,
            tc.tile_pool(name="wlp", bufs=16) as wlp,
            tc.tile_pool(name="hmp", bufs=hm_bufs) as hmp,
            tc.tile_pool(name="sgp", bufs=sg_bufs) as sgp,
            tc.tile_pool(name="yop", bufs=6) as yop,
            # split PSUM pools: gate/up accumulators vs down-proj py tiles,
            # so a psg allocation never WAR-waits on a py drain (and vice
            # versa) -- 4 + 4 banks
            tc.tile_pool(name="psA", bufs=4, space="PSUM") as psA,
            tc.tile_pool(name="psB", bufs=4, space="PSUM") as psB,
        ):
            # Resident x and y accumulators.  x DMAs are NOT emitted here:
            # DMA queues drain in emission order, so the first gate group's
            # weight tiles must interleave with x (see emit_gate_up first=True)
            # or the first matmul waits behind all 4MB of x.
            xq = []
            xq_pending = []
            for q in range(NXQ):
                xt = xp.tile([P, HQ, T], mm_dt, name=f"x{q}", tag=f"x{q}")
                xq.append(xt)
                xq_pending.append(q)
            yt = []
            for tt in range(TT):
                ytile = yp.tile([P, H], F32, name=f"y{tt}", tag=f"y{tt}")
                nc.vector.memset(ytile, 0.0)
                yt.append(ytile)

            hm_store = {}  # chunk idx -> list of hm tiles (one per subtile)

            def emit_gate_up(ci, s, first=False):
                """gate+up+silu+mul for one 128-wide i-subtile s."""
                gts = []
                uts0 = []
                for w in range(4):
                    gt = wgp.tile([P, WQ, P], mm_dt, tag="wg")
                    nc.sync.dma_start(out=gt, in_=wg_d[s, w])
                    gts.append(gt)
                    if first:
                        # need-ordered startup: x splits land between wg tiles
                        for q in range(w * NXQ // 4, (w + 1) * NXQ // 4):
                            nc.sync.dma_start(out=xq[q], in_=x_d[q])
                            xq_pending.remove(q)
                        # early wu quarters too, so psu never bunch-waits
                        ut = wup.tile([P, WQ, P], mm_dt, tag="wu")
                        nc.sync.dma_start(out=ut, in_=wu_d[s, w])
                        uts0.append(ut)
                psg = psA.tile([P, T], F32, tag="psA", name="psg")
                for hs in range(HS):
                    nc.tensor.matmul(
                        psg, gts[hs // WQ][:, hs % WQ, :],
                        xq[hs // HQ][:, hs % HQ, :],
                        start=hs == 0, stop=hs == HS - 1,
                    )
                if first:
                    uts = uts0
                else:
                    uts = []
                    for w in range(4):
                        ut = wup.tile([P, WQ, P], mm_dt, tag="wu")
                        nc.sync.dma_start(out=ut, in_=wu_d[s, w])
                        uts.append(ut)
                psu = psA.tile([P, T], F32, tag="psA", name="psu")
                for hs in range(HS):
                    nc.tensor.matmul(
                        psu, uts[hs // WQ][:, hs % WQ, :],
                        xq[hs // HQ][:, hs % HQ, :],
                        start=hs == 0, stop=hs == HS - 1,
                    )
                sg = sgp.tile([P, T], F32, tag="sg")
                if use_silu:
                    nc.scalar.activation(
                        sg, psg, mybir.ActivationFunctionType.Silu
                    )
                else:
                    # CoreSim lacks Silu: sigmoid + extra DVE mul
                    nc.scalar.activation(
                        sg, psg, mybir.ActivationFunctionType.Sigmoid
                    )
                    nc.vector.tensor_mul(sg, sg, psg)
                hm = hmp.tile([P, T], mm_dt, tag="hm")
                nc.vector.tensor_mul(hm, sg, psu)
                hm_store[ci].append(hm)

            def emit_down_block(pc, osc, wdts=None, final=False):
                """down-proj contribution of chunk pc to o-chunk osc."""
                subs = chunks[pc]
                hms = hm_store[pc]
                if wdts is None:
                    wdts = []
                    for s in subs:
                        wdt = wdp.tile([P, OCW], mm_dt, tag="wd")
                        nc.sync.dma_start(out=wdt, in_=wd_d[s, osc])
                        wdts.append(wdt)
                osl = slice(osc * OCW, (osc + 1) * OCW)
                n = len(subs)
                for tt in range(TT):
                    py = psB.tile([P, OCW], F32, tag="psB", name="py")
                    for k in range(n):
                        nc.tensor.matmul(
                            py, hms[k][:, tt * P:(tt + 1) * P], wdts[k],
                            start=k == 0, stop=k == n - 1,
                        )
                    if final:
                        # last contribution: add straight into the (narrower)
                        # output-dtype staging tile and ship it
                        yo = yop.tile([P, OCW], y_dt, tag="yo")
                        nc.vector.tensor_add(yo, yt[tt][:, osl], py)
                        nc.sync.dma_start(out=y_d[tt, osc], in_=yo)
                    else:
                        nc.vector.tensor_add(yt[tt][:, osl], yt[tt][:, osl], py)

            trail_wd = None
            for ci in range(NCH):
                hm_store[ci] = []
                its = chunks[ci]
                nits = len(its)
                for k, s in enumerate(its):
                    emit_gate_up(ci, s, first=(ci == 0 and k == 0))
                    if ci == NCH - 1 and k == 0:
                        # prefetch the final blocks' wd tiles now so the
                        # last-chunk down blocks never wait on DMA
                        trail_wd = []
                        for osc in range(NO):
                            tts_ = []
                            for s2 in chunks[NCH - 1]:
                                wdt = wlp.tile([P, OCW], mm_dt, tag="wl")
                                nc.sync.dma_start(out=wdt, in_=wd_d[s2, osc])
                                tts_.append(wdt)
                            trail_wd.append(tts_)
                    prev_oscs = (
                        list(range(k * NO // nits, (k + 1) * NO // nits))
                        if ci > 0 else []
                    )
                    if ci == NCH - 1 and k == nits - 1:
                        # last section: alternate the prev chunk's remaining
                        # blocks with this chunk's own (final) blocks, so the
                        # writeback DVE+DMA spreads over the whole section
                        # instead of bunching at the very end.  Lead with two
                        # prev blocks to give this chunk's last hm time to land.
                        own = list(range(NO))
                        for j in range(max(len(prev_oscs), len(own)) + 1):
                            if j < len(prev_oscs):
                                emit_down_block(ci - 1, prev_oscs[j])
                            if 0 < j <= len(own):
                                emit_down_block(
                                    ci, own[j - 1], wdts=trail_wd[own[j - 1]],
                                    final=True,
                                )
                    else:
                        for osc in prev_oscs:
                            emit_down_block(ci - 1, osc)
                if ci > 1:
                    hm_store.pop(ci - 2, None)

    nc.compile()
    return nc


def prep_weights(Wg, Wu, Wd, dt_np):
    """Host-side re-tiling of the weights into the device DMA layouts."""
    Iin, Hh = Wg.shape
    HS = Hh // P
    NO = Hh // OCW
    NSUB = math.ceil(Iin / P)
    IPAD = NSUB * P

    if IPAD != Iin:
        Wg_p = np.zeros((IPAD, Hh), np.float32)
        Wg_p[:Iin] = Wg
        Wu_p = np.zeros((IPAD, Hh), np.float32)
        Wu_p[:Iin] = Wu
        Wd_p = np.zeros((Hh, IPAD), np.float32)
        Wd_p[:, :Iin] = Wd
    else:
        Wg_p, Wu_p, Wd_p = Wg, Wu, Wd

    # wg[s, w, p, wq, ii] = Wg_p[s*128 + ii, (w*WQ + wq)*128 + p]
    WQ = HS // 4
    wg_host = Wg_p.reshape(NSUB, P, 4, WQ, P).transpose(0, 2, 4, 3, 1).astype(dt_np)
    wu_host = Wu_p.reshape(NSUB, P, 4, WQ, P).transpose(0, 2, 4, 3, 1).astype(dt_np)
    # wd[s, osc, p, oo] = Wd_p[osc*512 + oo, s*128 + p]
    wd_host = Wd_p.reshape(NO, OCW, NSUB, P).transpose(2, 0, 3, 1).astype(dt_np)
    return wg_host, wu_host, wd_host


def prep_x_shard(x2, c, T, dt_np):
    """x2 [tokens, H] -> core c's [NXQ, 128, HS/NXQ, T] split-tile layout."""
    Hh = x2.shape[1]
    HS = Hh // P
    NXQ = 8 if HS % 8 == 0 else 4
    HQ = HS // NXQ
    xs = x2[c * T:(c + 1) * T]  # [T, H]
    # x[q, p, hq, t] = xs[t, (q*HQ + hq)*128 + p]
    return xs.reshape(T, NXQ, HQ, P).transpose(1, 3, 2, 0).astype(dt_np)


def run_on_cores(nc, in_maps, **kwargs):
    return run_bass_kernel_spmd(nc, in_maps, core_ids=list(range(len(in_maps))), **kwargs)


_NC_CACHE = {}

# matmul dtype mode: "bf16" (1 PE cycle/row, half DMA, ~2e-3 rel err),
# "f32r" (tf32-like, 1 cycle/row, ~2e-4), "f32" (exact, 4 cycles/row)
MM_MODE = "bf16"
_MM_DT = {"bf16": BF16, "f32r": F32R, "f32": F32}


def _get_nc(mode=None):
    mode = mode or MM_MODE
    key = (T, H, I, mode)
    if key not in _NC_CACHE:
        _NC_CACHE[key] = build_nc(T, H, I, mm_dt=_MM_DT[mode])
    return _NC_CACHE[key]


def kernel(x, Wg, Wu, Wd, _trace=False, _trace_kwargs=None, _mode=None):
    x = np.asarray(x, np.float32)
    Wg = np.asarray(Wg, np.float32)
    Wu = np.asarray(Wu, np.float32)
    Wd = np.asarray(Wd, np.float32)

    mode = _mode or MM_MODE
    nc = _get_nc(mode)
    dt_np = mybir.dt.np(_MM_DT[mode])
    wg_host, wu_host, wd_host = prep_weights(Wg, Wu, Wd, dt_np)
    x2 = x.reshape(B * S, H)
    in_maps = [
        {
            "x": prep_x_shard(x2, c, T, dt_np),
            "wg": wg_host,
            "wu": wu_host,
            "wd": wd_host,
        }
        for c in range(NCORES)
    ]
    kwargs = {}
    if _trace:
        kwargs["trace"] = True
        kwargs.update(_trace_kwargs or {})
    res = run_on_cores(nc, in_maps, **kwargs)
    # y arrives as [TT, NO, 128, 512]: y[t, h] = y_d[t//128, h//512, t%128, h%512]
    shards = [
        np.asarray(res.results[c]["y"], np.float32)
        .transpose(0, 2, 1, 3)
        .reshape(T, H)
        for c in range(NCORES)
    ]
    y = np.concatenate(shards, axis=0).reshape(B, S, H)
    if _trace:
        return y, res
    return y


# revision 20
# speedup vs baseline: 1.1950x; 1.1950x over previous
"""Fused SwiGLU MLP (gate/up/down) Trainium2 Bass kernel.

Problem: y = down( silu(x @ Wg^T) * (x @ Wu^T) ) with
  x  [B=2, S=2048, H=4096]  f32
  Wg [I=11008, H]           f32   (gate proj, [out,in])
  Wu [I=11008, H]           f32
  Wd [H, I]                 f32

Strategy: data-parallel over tokens across the 8 NeuronCores.
Each core gets T = 4096/8 = 512 tokens and the full (replicated) weights,
computing the entire MLP for its token shard.  No collectives; the host
just concatenates the 8 token shards.

Per-core PE floor (f32r/bf16 at 1 cycle/row, 2.4 GHz):
  gate/up: 86 subtiles x 64 matmuls x 512 rows = 2.818M rows
  down:    86 x 8 osc x 4 tt matmuls x 512 rows = 1.409M rows
  total 4.227M rows x 0.4167ns = 1.761 ms.

v2 changes vs the 2.126 ms v1:
  * bf16 weights + activations (same 1 cyc/row PE rate as f32r, half the
    HBM traffic: 281 MB vs 562 MB, ~2e-3 rel err vs 2e-4 -- both well
    inside the 2e-2 gate).
  * i-subtile-staggered gate/up: accumulate psg[s] fully (32 h matmuls),
    then psu[s], so only 1-2 PSUM banks accumulate at a time (v1 held
    all 8 banks, forcing the PE to drain-stall at every chunk boundary).
  * down-proj of chunk ci-1 interleaved between the gate/up groups of
    chunk ci, so the PE stream never waits on ACT/DVE: by the time a
    down block issues, its hm inputs closed >20us earlier.
  * no I-padding waste: 11008 = 86 x 128 exactly (v1 padded to 11264).
  * y written back per-osc during the trailing down blocks (no 8MB tail).

Device layouts (host-side numpy retile, every DMA partition-major):
  x    [4, 128, HS/4, T]     quarter q, partition p=h%128, h-sub, token
  wg/wu[NSUB, 128, HS, 128]  subtile s: [p=h%128, hs, i%128]
  wd   [NSUB, NO, 128, 512]  [p=i%128, o%512]
  y    [TT, 128, H]          y[tt*128+p, o]
"""

import math

import numpy as np

import concourse.bass as bass
import concourse.mybir as mybir
import concourse.tile as tile
from concourse import bacc
from concourse.bass_utils import run_bass_kernel_spmd

F32 = mybir.dt.float32
F32R = mybir.dt.float32r
BF16 = mybir.dt.bfloat16
P = 128
OCW = 512  # o-chunk width (one PSUM bank of f32)

# full-size problem constants
B, S, H, I = 2, 2048, 4096, 11008
NCORES = 8
T = (B * S) // NCORES  # 512 tokens per core


def _chunks(nsub, width=4):
    return [list(range(a, min(a + width, nsub))) for a in range(0, nsub, width)]


def build_nc(T, H, I_valid, wg_bufs=10, wd_bufs=12, hm_bufs=9, sg_bufs=2,
             mm_dt=BF16, use_silu=True):
    assert T % P == 0 and T <= 512
    HS = H // P          # h subtiles (contraction for gate/up)
    assert H % OCW == 0 and HS % 4 == 0
    NO = H // OCW        # o chunks
    TT = T // P          # token tiles
    NSUB = math.ceil(I_valid / P)
    IPAD = NSUB * P
    chunks = _chunks(NSUB)
    NCH = len(chunks)
    NXQ = 8 if HS % 8 == 0 else 4  # x split (finer => earlier first matmul)
    HQ = HS // NXQ       # h subtiles per x split tile
    WQ = HS // 4         # h subtiles per wg/wu split tile

    nc = bacc.Bacc("TRN2", target_bir_lowering=False, debug=False)
    x_d = nc.dram_tensor("x", [NXQ, P, HQ, T], mm_dt, kind="ExternalInput").ap()
    wg_d = nc.dram_tensor("wg", [NSUB, 4, P, WQ, P], mm_dt, kind="ExternalInput").ap()
    wu_d = nc.dram_tensor("wu", [NSUB, 4, P, WQ, P], mm_dt, kind="ExternalInput").ap()
    wd_d = nc.dram_tensor("wd", [NSUB, NO, P, OCW], mm_dt, kind="ExternalInput").ap()
    # y leaves in matmul dtype (bf16 halves the writeback tail); host upcasts
    y_dt = mm_dt if mm_dt == BF16 else F32
    y_d = nc.dram_tensor("y", [TT, NO, P, OCW], y_dt, kind="ExternalOutput").ap()

    with tile.TileContext(nc) as tc:
        with (
            tc.tile_pool(name="xp", bufs=1) as xp,
            tc.tile_pool(name="yp", bufs=1) as yp,
            tc.tile_pool(name="wgp", bufs=wg_bufs) as wgp,
            tc.tile_pool(name="wup", bufs=wg_bufs) as wup,
            tc.tile_pool(name="wdp", bufs=wd_bufs) as wdp,
            tc.tile_pool(name="wlp", bufs=16) as wlp,
            tc.tile_pool(name="hmp", bufs=hm_bufs) as hmp,
            tc.tile_pool(name="sgp", bufs=sg_bufs) as sgp,
            tc.tile_pool(name="yop", bufs=6) as yop,
            # split PSUM pools: gate/up accumulators vs down-proj py tiles,
            # so a psg allocation never WAR-waits on a py drain (and vice
            # versa) -- 4 + 4 banks
            tc.tile_pool(name="psA", bufs=4, space="PSUM") as psA,
            tc.tile_pool(name="psB", bufs=4, space="PSUM") as psB,
        ):
            # Resident x and y accumulators.  x DMAs are NOT emitted here:
            # DMA queues drain in emission order, so the first gate group's
            # weight tiles must interleave with x (see emit_gate_up first=True)
            # or the first matmul waits behind all 4MB of x.
            xq = []
            xq_pending = []
            for q in range(NXQ):
                xt = xp.tile([P, HQ, T], mm_dt, name=f"x{q}", tag=f"x{q}")
                xq.append(xt)
                xq_pending.append(q)
            yt = []
            for tt in range(TT):
                ytile = yp.tile([P, H], F32, name=f"y{tt}", tag=f"y{tt}")
                nc.vector.memset(ytile, 0.0)
                yt.append(ytile)

            hm_store = {}  # chunk idx -> list of hm tiles (one per subtile)

            def emit_gate_up(ci, s, first=False):
                """gate+up+silu+mul for one 128-wide i-subtile s."""
                gts = []
                for w in range(4):
                    gt = wgp.tile([P, WQ, P], mm_dt, tag="wg")
                    nc.sync.dma_start(out=gt, in_=wg_d[s, w])
                    gts.append(gt)
                    if first:
                        # need-ordered startup: x splits land between wg tiles
                        for q in range(w * NXQ // 4, (w + 1) * NXQ // 4):
                            nc.sync.dma_start(out=xq[q], in_=x_d[q])
                            xq_pending.remove(q)
                psg = psA.tile([P, T], F32, tag="psA", name="psg")
                for hs in range(HS):
                    nc.tensor.matmul(
                        psg, gts[hs // WQ][:, hs % WQ, :],
                        xq[hs // HQ][:, hs % HQ, :],
                        start=hs == 0, stop=hs == HS - 1,
                    )
                uts = []
                for w in range(4):
                    ut = wup.tile([P, WQ, P], mm_dt, tag="wu")
                    nc.sync.dma_start(out=ut, in_=wu_d[s, w])
                    uts.append(ut)
                psu = psA.tile([P, T], F32, tag="psA", name="psu")
                for hs in range(HS):
                    nc.tensor.matmul(
                        psu, uts[hs // WQ][:, hs % WQ, :],
                        xq[hs // HQ][:, hs % HQ, :],
                        start=hs == 0, stop=hs == HS - 1,
                    )
                sg = sgp.tile([P, T], F32, tag="sg")
                if use_silu:
                    nc.scalar.activation(
                        sg, psg, mybir.ActivationFunctionType.Silu
                    )
                else:
                    # CoreSim lacks Silu: sigmoid + extra DVE mul
                    nc.scalar.activation(
                        sg, psg, mybir.ActivationFunctionType.Sigmoid
                    )
                    nc.vector.tensor_mul(sg, sg, psg)
                hm = hmp.tile([P, T], mm_dt, tag="hm")
                nc.vector.tensor_mul(hm, sg, psu)
                hm_store[ci].append(hm)

            def emit_down_block(pc, osc, wdts=None, final=False):
                """down-proj contribution of chunk pc to o-chunk osc."""
                subs = chunks[pc]
                hms = hm_store[pc]
                if wdts is None:
                    wdts = []
                    for s in subs:
                        wdt = wdp.tile([P, OCW], mm_dt, tag="wd")
                        nc.sync.dma_start(out=wdt, in_=wd_d[s, osc])
                        wdts.append(wdt)
                osl = slice(osc * OCW, (osc + 1) * OCW)
                n = len(subs)
                for tt in range(TT):
                    py = psB.tile([P, OCW], F32, tag="psB", name="py")
                    for k in range(n):
                        nc.tensor.matmul(
                            py, hms[k][:, tt * P:(tt + 1) * P], wdts[k],
                            start=k == 0, stop=k == n - 1,
                        )
                    if final:
                        # last contribution: add straight into the (narrower)
                        # output-dtype staging tile and ship it
                        yo = yop.tile([P, OCW], y_dt, tag="yo")
                        nc.vector.tensor_add(yo, yt[tt][:, osl], py)
                        nc.sync.dma_start(out=y_d[tt, osc], in_=yo)
                    else:
                        nc.vector.tensor_add(yt[tt][:, osl], yt[tt][:, osl], py)

            trail_wd = None
            for ci in range(NCH):
                hm_store[ci] = []
                its = chunks[ci]
                nits = len(its)
                for k, s in enumerate(its):
                    emit_gate_up(ci, s, first=(ci == 0 and k == 0))
                    if ci == NCH - 1 and k == 0:
                        # prefetch the final blocks' wd tiles now so the
                        # last-chunk down blocks never wait on DMA
                        trail_wd = []
                        for osc in range(NO):
                            tts_ = []
                            for s2 in chunks[NCH - 1]:
                                wdt = wlp.tile([P, OCW], mm_dt, tag="wl")
                                nc.sync.dma_start(out=wdt, in_=wd_d[s2, osc])
                                tts_.append(wdt)
                            trail_wd.append(tts_)
                    prev_oscs = (
                        list(range(k * NO // nits, (k + 1) * NO // nits))
                        if ci > 0 else []
                    )
                    if ci == NCH - 1 and k == nits - 1:
                        # last section: alternate the prev chunk's remaining
                        # blocks with this chunk's own (final) blocks, so the
                        # writeback DVE+DMA spreads over the whole section
                        # instead of bunching at the very end.  Lead with two
                        # prev blocks to give this chunk's last hm time to land.
                        own = list(range(NO))
                        for j in range(max(len(prev_oscs), len(own)) + 1):
                            if j < len(prev_oscs):
                                emit_down_block(ci - 1, prev_oscs[j])
                            if 0 < j <= len(own):
                                emit_down_block(
                                    ci, own[j - 1], wdts=trail_wd[own[j - 1]],
                                    final=True,
                                )
                    else:
                        for osc in prev_oscs:
                            emit_down_block(ci - 1, osc)
                if ci > 1:
                    hm_store.pop(ci - 2, None)

    nc.compile()
    return nc


def prep_weights(Wg, Wu, Wd, dt_np):
    """Host-side re-tiling of the weights into the device DMA layouts."""
    Iin, Hh = Wg.shape
    HS = Hh // P
    NO = Hh // OCW
    NSUB = math.ceil(Iin / P)
    IPAD = NSUB * P

    if IPAD != Iin:
        Wg_p = np.zeros((IPAD, Hh), np.float32)
        Wg_p[:Iin] = Wg
        Wu_p = np.zeros((IPAD, Hh), np.float32)
        Wu_p[:Iin] = Wu
        Wd_p = np.zeros((Hh, IPAD), np.float32)
        Wd_p[:, :Iin] = Wd
    else:
        Wg_p, Wu_p, Wd_p = Wg, Wu, Wd

    # wg[s, w, p, wq, ii] = Wg_p[s*128 + ii, (w*WQ + wq)*128 + p]
    WQ = HS // 4
    wg_host = Wg_p.reshape(NSUB, P, 4, WQ, P).transpose(0, 2, 4, 3, 1).astype(dt_np)
    wu_host = Wu_p.reshape(NSUB, P, 4, WQ, P).transpose(0, 2, 4, 3, 1).astype(dt_np)
    # wd[s, osc, p, oo] = Wd_p[osc*512 + oo, s*128 + p]
    wd_host = Wd_p.reshape(NO, OCW, NSUB, P).transpose(2, 0, 3, 1).astype(dt_np)
    return wg_host, wu_host, wd_host


def prep_x_shard(x2, c, T, dt_np):
    """x2 [tokens, H] -> core c's [NXQ, 128, HS/NXQ, T] split-tile layout."""
    Hh = x2.shape[1]
    HS = Hh // P
    NXQ = 8 if HS % 8 == 0 else 4
    HQ = HS // NXQ
    xs = x2[c * T:(c + 1) * T]  # [T, H]
    # x[q, p, hq, t] = xs[t, (q*HQ + hq)*128 + p]
    return xs.reshape(T, NXQ, HQ, P).transpose(1, 3, 2, 0).astype(dt_np)


def run_on_cores(nc, in_maps, **kwargs):
    return run_bass_kernel_spmd(nc, in_maps, core_ids=list(range(len(in_maps))), **kwargs)


_NC_CACHE = {}

# matmul dtype mode: "bf16" (1 PE cycle/row, half DMA, ~2e-3 rel err),
# "f32r" (tf32-like, 1 cycle/row, ~2e-4), "f32" (exact, 4 cycles/row)
MM_MODE = "bf16"
_MM_DT = {"bf16": BF16, "f32r": F32R, "f32": F32}


def _get_nc(mode=None):
    mode = mode or MM_MODE
    key = (T, H, I, mode)
    if key not in _NC_CACHE:
        _NC_CACHE[key] = build_nc(T, H, I, mm_dt=_MM_DT[mode])
    return _NC_CACHE[key]


def kernel(x, Wg, Wu, Wd, _trace=False, _trace_kwargs=None, _mode=None):
    x = np.asarray(x, np.float32)
    Wg = np.asarray(Wg, np.float32)
    Wu = np.asarray(Wu, np.float32)
    Wd = np.asarray(Wd, np.float32)

    mode = _mode or MM_MODE
    nc = _get_nc(mode)
    dt_np = mybir.dt.np(_MM_DT[mode])
    wg_host, wu_host, wd_host = prep_weights(Wg, Wu, Wd, dt_np)
    x2 = x.reshape(B * S, H)
    in_maps = [
        {
            "x": prep_x_shard(x2, c, T, dt_np),
            "wg": wg_host,
            "wu": wu_host,
            "wd": wd_host,
        }
        for c in range(NCORES)
    ]
    kwargs = {}
    if _trace:
        kwargs["trace"] = True
        kwargs.update(_trace_kwargs or {})
    res = run_on_cores(nc, in_maps, **kwargs)
    # y arrives as [TT, NO, 128, 512]: y[t, h] = y_d[t//128, h//512, t%128, h%512]
    shards = [
        np.asarray(res.results[c]["y"], np.float32)
        .transpose(0, 2, 1, 3)
        .reshape(T, H)
        for c in range(NCORES)
    ]
    y = np.concatenate(shards, axis=0).reshape(B, S, H)
    if _trace:
        return y, res
    return y
